# revision 8
# baseline (speedup 1.0000x reference)
"""CommAttention on 8 Trainium2 NeuronCores — head-parallel (core i owns head i).

Per core: grouped QKV projections (bf16 matmuls, f32 accum), per-(batch,head)
16x16 attention done on 8-batch groups via a 128x128 cross-product matmul with
block-diagonal masking, then the grouped output projection. Each core emits its
head's additive partial of the (B, NB, HID) output; the host sums the 8
partials.

Layouts (host-prepped):
  hT    (HID, NB, B)   bf16  — h transposed so hid is the matmul contraction dim
  wq/wk (NB, HID, KD)  bf16  — head slice, natural [K, M] for lhsT
  wv    (NB, HID, HID) bf16
  wo    (NB, HID, HID) bf16  — head's row-slice of Wo
  maskf (B*NB,)        f32   — mask flattened b-major
  mbd   (128, 128)     f32   — block-diagonal (8 blocks of 16x16 ones)
"""

import math
from contextlib import ExitStack

import numpy as np

B, NB, HID, KD, NH = 256, 16, 512, 64, 8
NCORES = 8
GB = 8            # batches per attention group
NG = B // GB      # 32 attention groups
P = 128
KC = HID // P     # 4 contraction chunks
BC = B // P       # 2 batch chunks of 128

_CACHE: dict = {}


def _build_bass():
    import concourse.bass as bass
    from concourse import bacc
    import concourse.tile as tile
    from concourse import mybir
    from concourse.masks import make_identity

    f32 = mybir.dt.float32
    bf16 = mybir.dt.bfloat16

    nc = bacc.Bacc()
    hT = nc.dram_tensor("hT", (HID, NB, B), bf16, kind="ExternalInput")
    wq = nc.dram_tensor("wq", (NB, HID, KD), bf16, kind="ExternalInput")
    wk = nc.dram_tensor("wk", (NB, HID, KD), bf16, kind="ExternalInput")
    wv = nc.dram_tensor("wv", (NB, HID, HID), bf16, kind="ExternalInput")
    wo = nc.dram_tensor("wo", (NB, HID, HID), bf16, kind="ExternalInput")
    maskf = nc.dram_tensor("maskf", (B * NB,), f32, kind="ExternalInput")
    mbd = nc.dram_tensor("mbd", (P, P), f32, kind="ExternalInput")
    out = nc.dram_tensor("out", (B, NB, HID), f32, kind="ExternalOutput")

    with ExitStack() as ctx:
        tc = ctx.enter_context(tile.TileContext(nc))
        singles = ctx.enter_context(tc.tile_pool(name="singles", bufs=1))
        wpool = ctx.enter_context(tc.tile_pool(name="wpool", bufs=3))
        work = ctx.enter_context(tc.tile_pool(name="work", bufs=3))
        psum = ctx.enter_context(tc.tile_pool(name="psum", bufs=6, space="PSUM"))
        psum2 = ctx.enter_context(tc.tile_pool(name="psum2", bufs=2, space="PSUM"))

        # ---- persistent SBUF state ----
        hTn = []
        for n in range(NB):
            t = singles.tile([P, KC, B], bf16, tag=f"ht{n}")
            nc.sync.dma_start(
                out=t, in_=hT[:, n, :].rearrange("(kc p) b -> p kc b", p=P)
            )
            hTn.append(t)
        qT_sb = singles.tile([KD, B, NB], bf16, tag="qT")
        kT_sb = singles.tile([KD, B, NB], bf16, tag="kT")
        v_sb = singles.tile([P, BC, NB, HID], bf16, tag="vsb")
        ctxT_sb = singles.tile([P, KC, B, NB], bf16, tag="ctxT")
        maskcol = singles.tile([P, NG], f32, tag="maskc")
        nc.sync.dma_start(out=maskcol, in_=maskf.rearrange("(g p) -> p g", p=P))
        mbd_sb = singles.tile([P, P], f32, tag="mbd")
        nc.sync.dma_start(out=mbd_sb, in_=mbd[:, :])
        ident = singles.tile([P, P], bf16, tag="ident")
        make_identity(nc, ident)

        # ---- Phase B: grouped QKV projections ----
        for n in range(NB):
            wq_t = wpool.tile([P, KC, KD], bf16, tag="wq")
            nc.sync.dma_start(out=wq_t, in_=wq[n].rearrange("(kc p) m -> p kc m", p=P))
            wk_t = wpool.tile([P, KC, KD], bf16, tag="wk")
            nc.sync.dma_start(out=wk_t, in_=wk[n].rearrange("(kc p) m -> p kc m", p=P))
            wv_t = wpool.tile([P, KC, HID], bf16, tag="wv")
            nc.sync.dma_start(out=wv_t, in_=wv[n].rearrange("(kc p) d -> p kc d", p=P))

            pq = psum.tile([P, 512], f32, tag="ps")
            for kc in range(KC):
                nc.tensor.matmul(
                    pq[:KD, :B], wq_t[:, kc, :], hTn[n][:, kc, :],
                    start=(kc == 0), stop=(kc == KC - 1),
                )
            nc.vector.tensor_copy(out=qT_sb[:, :, n], in_=pq[:KD, :B])

            pk = psum.tile([P, 512], f32, tag="ps")
            for kc in range(KC):
                nc.tensor.matmul(
                    pk[:KD, :B], wk_t[:, kc, :], hTn[n][:, kc, :],
                    start=(kc == 0), stop=(kc == KC - 1),
                )
            nc.vector.tensor_copy(out=kT_sb[:, :, n], in_=pk[:KD, :B])

            for bc in range(BC):
                pv = psum.tile([P, 512], f32, tag="ps")
                for kc in range(KC):
                    nc.tensor.matmul(
                        pv, hTn[n][:, kc, bc * P:(bc + 1) * P], wv_t[:, kc, :],
                        start=(kc == 0), stop=(kc == KC - 1),
                    )
                nc.vector.tensor_copy(out=v_sb[:, bc, n, :], in_=pv)

        # ---- Phase C/D: attention per 8-batch group ----
        for g in range(NG):
            bs = slice(g * GB, (g + 1) * GB)
            # gather V rows (b,k) for this group: [8b, 16k, 512] -> [128, 512]
            vpack = work.tile([P, HID], bf16, tag="vpack")
            p0 = (g * GB) % P
            nc.sync.dma_start(
                out=vpack, in_=v_sb[p0:p0 + GB, g // (P // GB), :, :]
            )

            ps_s = psum.tile([P, 512], f32, tag="ps")
            nc.tensor.matmul(
                ps_s[:, :P],
                qT_sb[:, bs, :].rearrange("p b q -> p (b q)"),
                kT_sb[:, bs, :].rearrange("p b k -> p (b k)"),
                start=True, stop=True,
            )
            exp_sb = work.tile([P, P], f32, tag="exp")
            nc.scalar.activation(
                out=exp_sb, in_=ps_s[:, :P],
                func=mybir.ActivationFunctionType.Exp,
                scale=1.0 / math.sqrt(KD),
            )
            # zero cross-batch blocks, then row-normalize
            nc.vector.tensor_mul(out=exp_sb, in0=exp_sb, in1=mbd_sb)
            rs = work.tile([P, 1], f32, tag="rs")
            nc.vector.reduce_sum(out=rs, in_=exp_sb, axis=mybir.AxisListType.X)
            nc.vector.reciprocal(out=rs, in_=rs)
            nc.vector.tensor_mul(out=rs, in0=rs, in1=maskcol[:, g:g + 1])
            probs = work.tile([P, P], bf16, tag="probs")
            nc.vector.tensor_scalar_mul(out=probs, in0=exp_sb, scalar1=rs)

            ps_t = psum2.tile([P, P], bf16, tag="pst")
            nc.tensor.transpose(ps_t, probs, ident)
            probsT = work.tile([P, P], bf16, tag="probsT")
            nc.vector.tensor_copy(out=probsT, in_=ps_t)

            ps_c = psum.tile([P, 512], f32, tag="ps")
            for dc in range(KC):
                nc.tensor.matmul(
                    ps_c[:, dc * P:(dc + 1) * P],
                    vpack[:, dc * P:(dc + 1) * P], probsT,
                    start=True, stop=True,
                )
            for dc in range(KC):
                nc.vector.tensor_copy(
                    out=ctxT_sb[:, dc, bs, :],
                    in_=ps_c[:, dc * P:(dc + 1) * P].rearrange(
                        "p (b q) -> p b q", b=GB
                    ),
                )

        # ---- Phase E: grouped output projection ----
        for n in range(NB):
            wo_t = wpool.tile([P, KC, HID], bf16, tag="wo")
            nc.sync.dma_start(out=wo_t, in_=wo[n].rearrange("(dc p) h -> p dc h", p=P))
            for bc in range(BC):
                po = psum.tile([P, 512], f32, tag="ps")
                for dc in range(KC):
                    nc.tensor.matmul(
                        po,
                        ctxT_sb[:, dc, bc * P:(bc + 1) * P, n],
                        wo_t[:, dc, :],
                        start=(dc == 0), stop=(dc == KC - 1),
                    )
                out_sb = work.tile([P, HID], f32, tag="osb")
                nc.vector.tensor_copy(out=out_sb, in_=po)
                nc.sync.dma_start(
                    out=out[bc * P:(bc + 1) * P, n, :], in_=out_sb
                )

    nc.finalize()
    return nc


def _get_compiled():
    if "nc" not in _CACHE:
        _CACHE["nc"] = _build_bass()
    return _CACHE["nc"]


def kernel(h, mask, Wk, Wq, Wv, Wo):
    import ml_dtypes
    from concourse.bass_utils import run_bass_kernel_spmd

    bf16 = ml_dtypes.bfloat16
    h = np.asarray(h, dtype=np.float32)
    mask = np.asarray(mask)
    Wk = np.asarray(Wk, dtype=np.float32)
    Wq = np.asarray(Wq, dtype=np.float32)
    Wv = np.asarray(Wv, dtype=np.float32)
    Wo = np.asarray(Wo, dtype=np.float32)

    hT = np.ascontiguousarray(h.transpose(2, 1, 0)).astype(bf16)
    maskf = np.ascontiguousarray(mask.astype(np.float32).reshape(B * NB))
    mbd = np.zeros((P, P), dtype=np.float32)
    for b in range(GB):
        mbd[b * NB:(b + 1) * NB, b * NB:(b + 1) * NB] = 1.0

    in_maps = []
    for i in range(NCORES):
        in_maps.append({
            "hT": hT,
            "wq": np.ascontiguousarray(Wq[:, :, i * KD:(i + 1) * KD]).astype(bf16),
            "wk": np.ascontiguousarray(Wk[:, :, i * KD:(i + 1) * KD]).astype(bf16),
            "wv": np.ascontiguousarray(Wv[:, :, i * HID:(i + 1) * HID]).astype(bf16),
            "wo": np.ascontiguousarray(Wo[:, i * HID:(i + 1) * HID, :]).astype(bf16),
            "maskf": maskf,
            "mbd": mbd,
        })

    nc = _get_compiled()
    res = run_bass_kernel_spmd(nc, in_maps, core_ids=list(range(NCORES)))
    _CACHE["last_results"] = res
    total = np.zeros((B, NB, HID), dtype=np.float32)
    for r in res.results:
        total += r["out"]
    return total


# revision 15
# speedup vs baseline: 1.4902x; 1.4902x over previous
"""CommAttention on 8 Trainium2 NeuronCores — head-parallel (core i owns head i).

Per core: grouped QKV projections (bf16 matmuls, f32 accum), per-(batch,head)
16x16 attention over 8-batch groups via 128x128 cross-product matmuls with
block-diagonal masking, then the grouped output projection. Each core emits its
head's additive partial of the (B, NB, HID) output; the host sums the 8
partials.

Attention trick: scoresT [bk, bq] -> exp (bf16, unnormalized) -> zero
cross-batch blocks -> colsum via ones-matmul -> ctxT = V.T @ expT
(unnormalized). The softmax 1/colsum (and the post-softmax row mask) is
applied at the very end as a per-partition scale on the output-projection
PSUM->SBUF copy, where (b,q) sits on partitions.

All DRAM inputs are host-packed partition-major so every SBUF load is a few
large contiguous descriptors:
  hT    (128, NB, KC, B)     bf16  h[b,n,kc*128+p] -> [p,n,kc,b]
  wqkv  (NB, 128, KC, 640)   bf16  concat(wq 64 | wk 64 | wv 512) cols
  wo    (NB, 128, KC, HID)   bf16
  mbd   (128, 128)           bf16  block-diagonal (8 blocks of 16x16 ones)
  maskE (128, BC, NB)        f32   mask[bc*128+p, n]
Output: (BC, 128, NB, HID) bf16 partial, summed in f32 on host.
"""

import math
from contextlib import ExitStack

import numpy as np

B, NB, HID, KD, NH = 256, 16, 512, 64, 8
NCORES = 8
GB = 8            # batches per attention group
NG = B // GB      # 32 attention groups
GPC = 16          # groups per batch-chunk
P = 128
KC = HID // P     # 4 contraction chunks
BC = B // P       # 2 batch chunks of 128
WQKV = KD + KD + HID  # 640 fused projection output cols

_CACHE: dict = {}


def _build_bass():
    import concourse.tile as tile
    from concourse import bacc, mybir

    f32 = mybir.dt.float32
    bf16 = mybir.dt.bfloat16

    nc = bacc.Bacc()
    hT = nc.dram_tensor("hT", (P, NB, KC, B), bf16, kind="ExternalInput")
    wqkv = nc.dram_tensor("wqkv", (NB, P, KC, WQKV), bf16, kind="ExternalInput")
    wo = nc.dram_tensor("wo", (NB, P, KC, HID), bf16, kind="ExternalInput")
    mbd = nc.dram_tensor("mbd", (P, P), bf16, kind="ExternalInput")
    maskE = nc.dram_tensor("maskE", (P, BC, NB), f32, kind="ExternalInput")
    out = nc.dram_tensor("out", (BC, P, NB, HID), bf16, kind="ExternalOutput")

    with ExitStack() as ctx:
        tc = ctx.enter_context(tile.TileContext(nc))
        singles = ctx.enter_context(tc.tile_pool(name="singles", bufs=1))
        wpool = ctx.enter_context(tc.tile_pool(name="wpool", bufs=3))
        work = ctx.enter_context(tc.tile_pool(name="work", bufs=3))
        psum = ctx.enter_context(tc.tile_pool(name="psum", bufs=6, space="PSUM"))
        psum2 = ctx.enter_context(tc.tile_pool(name="psum2", bufs=2, space="PSUM"))

        # ---- persistent SBUF state ----
        hT_sb = singles.tile([P, NB, KC, B], bf16, tag="ht")
        for c in range(4):
            ns = slice(c * 4, (c + 1) * 4)
            nc.sync.dma_start(out=hT_sb[:, ns], in_=hT[:, ns])
        qT_sb = singles.tile([KD, B, NB], bf16, tag="qT")
        kT_sb = singles.tile([KD, B, NB], bf16, tag="kT")
        v_t = [
            singles.tile([P, NB, HID], bf16, tag=f"vsb{bc}", name=f"vsb{bc}")
            for bc in range(BC)
        ]
        ctxT_t = [
            singles.tile([P, KC, P, NB], bf16, tag=f"ctxT{bc}", name=f"ctxT{bc}")
            for bc in range(BC)
        ]
        cs_t = [
            singles.tile([1, GPC * P], f32, tag=f"cs{bc}", name=f"cs{bc}")
            for bc in range(BC)
        ]
        facE = singles.tile([P, BC, NB], f32, tag="facE")
        maskE_sb = singles.tile([P, BC, NB], f32, tag="maskE")
        nc.sync.dma_start(out=maskE_sb, in_=maskE[:, :, :])
        mbd_sb = singles.tile([P, P], bf16, tag="mbd")
        nc.sync.dma_start(out=mbd_sb, in_=mbd[:, :])
        ones_sb = singles.tile([P, 1], bf16, tag="ones")
        nc.vector.memset(ones_sb, 1.0)

        # ---- Phase B: grouped QKV projections ----
        for n in range(NB):
            w_t = wpool.tile([P, KC, WQKV], bf16, tag="wqkv")
            nc.sync.dma_start(out=w_t, in_=wqkv[n])

            pq = psum.tile([P, 512], f32, tag="ps")
            for kc in range(KC):
                nc.tensor.matmul(
                    pq[:KD, :B], w_t[:, kc, 0:KD], hT_sb[:, n, kc, :],
                    start=(kc == 0), stop=(kc == KC - 1),
                )
            nc.scalar.copy(out=qT_sb[:, :, n], in_=pq[:KD, :B])

            pk = psum.tile([P, 512], f32, tag="ps")
            for kc in range(KC):
                nc.tensor.matmul(
                    pk[:KD, :B], w_t[:, kc, KD:2 * KD], hT_sb[:, n, kc, :],
                    start=(kc == 0), stop=(kc == KC - 1),
                )
            nc.scalar.copy(out=kT_sb[:, :, n], in_=pk[:KD, :B])

            for bc in range(BC):
                pv = psum.tile([P, 512], f32, tag="ps")
                for kc in range(KC):
                    nc.tensor.matmul(
                        pv, hT_sb[:, n, kc, bc * P:(bc + 1) * P],
                        w_t[:, kc, 2 * KD:WQKV],
                        start=(kc == 0), stop=(kc == KC - 1),
                    )
                nc.vector.tensor_copy(out=v_t[bc][:, n, :], in_=pv)

        # ---- Phase C/D: attention per 8-batch group ----
        for g in range(NG):
            bc, gi = g // GPC, g % GPC
            bs = slice(g * GB, (g + 1) * GB)
            # gather V rows (b,k) for this group: [8b, 16k, 512] -> [128, 512]
            vpack = work.tile([P, HID], bf16, tag="vpack")
            p0 = gi * GB
            nc.gpsimd.dma_start(out=vpack, in_=v_t[bc][p0:p0 + GB, :, :])

            ps_s = psum.tile([P, 512], f32, tag="ps")
            nc.tensor.matmul(
                ps_s[:, :P],
                kT_sb[:, bs, :].rearrange("p b k -> p (b k)"),
                qT_sb[:, bs, :].rearrange("p b q -> p (b q)"),
                start=True, stop=True,
            )
            # unnormalized exp, bf16; then zero the cross-batch blocks
            expT = work.tile([P, P], bf16, tag="expT")
            nc.scalar.activation(
                out=expT, in_=ps_s[:, :P],
                func=mybir.ActivationFunctionType.Exp,
                scale=1.0 / math.sqrt(KD),
            )
            nc.vector.tensor_mul(out=expT, in0=expT, in1=mbd_sb)

            # per-(b,q) softmax denominators via ones-matmul (sum over bk)
            ps_cs = psum2.tile([1, P], f32, tag="pcs")
            nc.tensor.matmul(ps_cs, ones_sb, expT, start=True, stop=True)
            nc.scalar.copy(out=cs_t[bc][:, gi * P:(gi + 1) * P], in_=ps_cs)

            # ctxT_raw[d, bq] = sum_bk V[bk, d] * expT[bk, bq]
            ps_c = psum.tile([P, 512], f32, tag="ps")
            for dc in range(KC):
                nc.tensor.matmul(
                    ps_c[:, dc * P:(dc + 1) * P],
                    vpack[:, dc * P:(dc + 1) * P], expT,
                    start=True, stop=True,
                )
            nc.vector.tensor_copy(
                out=ctxT_t[bc][:, :, p0:p0 + GB, :],
                in_=ps_c.rearrange("p (dc b q) -> p dc b q", dc=KC, b=GB),
            )

        # ---- normalization factors: mask / colsum, rearranged to [b, n] ----
        for bc in range(BC):
            nc.gpsimd.dma_start(out=facE[:, bc, :], in_=cs_t[bc])
        nc.vector.reciprocal(out=facE, in_=facE)
        nc.vector.tensor_mul(out=facE, in0=facE, in1=maskE_sb)

        # ---- Phase E: grouped output projection (2 positions per store) ----
        for n0 in range(0, NB, 2):
            wo_t = wpool.tile([P, 2, KC, HID], bf16, tag="wo")
            nc.sync.dma_start(out=wo_t, in_=wo[n0:n0 + 2].rearrange("j p dc h -> p j dc h"))
            for bc in range(BC):
                out_sb = work.tile([P, 2, HID], bf16, tag="osb")
                for j in range(2):
                    n = n0 + j
                    po = psum.tile([P, 512], f32, tag="ps")
                    for dc in range(KC):
                        nc.tensor.matmul(
                            po,
                            ctxT_t[bc][:, dc, :, n],
                            wo_t[:, j, dc, :],
                            start=(dc == 0), stop=(dc == KC - 1),
                        )
                    # apply softmax 1/colsum and the post-softmax row mask
                    nc.scalar.activation(
                        out=out_sb[:, j, :], in_=po,
                        func=mybir.ActivationFunctionType.Copy,
                        scale=facE[:, bc, n:n + 1],
                    )
                nc.gpsimd.dma_start(out=out[bc, :, n0:n0 + 2, :], in_=out_sb)

    nc.finalize()
    return nc


def _get_compiled():
    if "nc" not in _CACHE:
        _CACHE["nc"] = _build_bass()
    return _CACHE["nc"]


def kernel(h, mask, Wk, Wq, Wv, Wo):
    import ml_dtypes
    from concourse.bass_utils import run_bass_kernel_spmd

    bf16 = ml_dtypes.bfloat16
    h = np.asarray(h, dtype=np.float32)
    mask = np.asarray(mask)
    Wk = np.asarray(Wk, dtype=np.float32)
    Wq = np.asarray(Wq, dtype=np.float32)
    Wv = np.asarray(Wv, dtype=np.float32)
    Wo = np.asarray(Wo, dtype=np.float32)

    # h[b, n, kc*128+p] -> hT[p, n, kc, b]
    hT = np.ascontiguousarray(
        h.reshape(B, NB, KC, P).transpose(3, 1, 2, 0)
    ).astype(bf16)
    maskE = np.ascontiguousarray(
        mask.astype(np.float32).reshape(BC, P, NB).transpose(1, 0, 2)
    )
    mbd = np.zeros((P, P), dtype=np.float32)
    for b in range(GB):
        mbd[b * NB:(b + 1) * NB, b * NB:(b + 1) * NB] = 1.0
    mbd = mbd.astype(bf16)

    in_maps = []
    for i in range(NCORES):
        wq_i = Wq[:, :, i * KD:(i + 1) * KD]          # (16, 512, 64)
        wk_i = Wk[:, :, i * KD:(i + 1) * KD]
        wv_i = Wv[:, :, i * HID:(i + 1) * HID]        # (16, 512, 512)
        wo_i = Wo[:, i * HID:(i + 1) * HID, :]        # (16, 512, 512)
        # (n, hid=kc*128+p, m) -> (n, p, kc, m); fuse q|k|v along m
        wqkv_i = np.concatenate([wq_i, wk_i, wv_i], axis=2)
        wqkv_i = np.ascontiguousarray(
            wqkv_i.reshape(NB, KC, P, WQKV).transpose(0, 2, 1, 3)
        ).astype(bf16)
        wo_p = np.ascontiguousarray(
            wo_i.reshape(NB, KC, P, HID).transpose(0, 2, 1, 3)
        ).astype(bf16)
        in_maps.append({
            "hT": hT,
            "wqkv": wqkv_i,
            "wo": wo_p,
            "mbd": mbd,
            "maskE": maskE,
        })

    nc = _get_compiled()
    res = run_bass_kernel_spmd(nc, in_maps, core_ids=list(range(NCORES)))
    _CACHE["last_results"] = res
    total = np.zeros((BC, P, NB, HID), dtype=np.float32)
    for r in res.results:
        total += r["out"].astype(np.float32)
    return np.ascontiguousarray(total.reshape(B, NB, HID))
